# revision 22
# baseline (speedup 1.0000x reference)
"""Trainium2 Bass kernel for nn_CircularConvolution_5403068858821.

The reference computes result[:, :, n] += 1 for m in range(M) -> a constant
tensor of shape [N, C, L_x + M - 1] filled with M (=16.0). The inputs are
never used arithmetically, so the optimal kernel is a pure HBM fill:
each of the 8 cores memsets an SBUF tile to 16.0 once and DMA-broadcasts
it over its shard of the output. No input bytes ever touch the device.

Sharding: data-parallel over batch N=32 -> 4 batches/core; per-core output
is [4*512, 4111] = [2048, 4111] f32 (~33.7 MB of HBM writes per core).

Critical-path trims over the first working kernel (97,681 ns model):
  - memset width 512 -> 176 f32 (the minimum whose bulk descriptors stay
    >= 512 B and whose 128-f32 remainder descriptor also stays >= 512 B,
    dodging the <512 B half-rate DMA penalty): memset exec 522 -> 242 ns.
    Real-HW slope check: 704 B descriptors fill at ~282 GB/s/core, no
    slower than the original 2 KB descriptors (~119 vs ~135 us/33.5 MB).
  - the vsem wait is attached to the bulk DMACopy itself (_wait_ge)
    instead of a separate EventSemaphore, so SEQ decode overlaps the
    memset. The wait itself must stay: it orders SBUF data vs the DMA's
    source reads (real race, only a few hundred ns of natural margin).
  - no final wait on the DMA-completion semaphore: each DMA still carries
    a sem increment (walrus codegen requires >=1 update per HWDGE DMA --
    omitting them SIGABRTs the birverifier), but nothing waits on it, so
    the post-wait exec/branch/drain/end-barrier (~310 ns) leave the
    critical path. The timeline still ends only at the last DMA's sem
    propagation (last byte + 900 ns). Verified on hardware: 12/12 runs
    with every one of the 538M output elements exact; host readback
    (PJRT round-trip, ms-scale) trails the 93.5 us ring drain by >1000x.
    A fully-synced vanilla program is kept as an automatic fallback.
  - the memset is emitted into the main (preamble) block, skipping the
    Pool engine's block-entry branch: -61 ns.
  - monotonic_sem_count=0 drops one preamble sem init: -61 ns.
  - IR surgery (_strip_preamble) removes the unused const-AP memsets,
    all RegisterMove scratch-register init, and the ENTIRE preamble
    barrier (vsem's initial-value guard is redundant -- see the
    _strip_preamble docstring); the end-of-block barrier is reduced to
    the Pool<->SP pair. Preamble ~780 ns -> ~60 ns.
  - split memset + two-DMA pipeline: the [0:128] region gating DMA A
    (512 B descriptors, first 1536 columns) is memset by Pool and DVE in
    parallel (split at column 90, balancing Pool's Q7 launch against
    DVE's access latency), releasing A ~130 ns earlier than one
    full-tile memset; DMA B's HWDGE generation and DGE delay hide under
    A's ~2.2 us transfer, so B starts with zero gap.
Model (TimelineSim) per core: 95,982 ns vs 93,548 ns pure-transfer floor;
the remaining gap is the dual-engine memset chain (~260 ns), HWDGE
generation (625 ns), the DGE->DMA delay (650 ns) and the trailing DMA
sem propagation (900 ns, hardware-real: the completion descriptor's
write-after-write dependency).
"""

import os
import time

import numpy as np

import concourse.bass as bass
import concourse.mybir as mybir
from concourse.bass_utils import run_bass_kernel_spmd

# Problem constants (hardcoded per the grading contract).
N, C, L_X = 32, 512, 4096
M = 16
L = L_X + M - 1  # 4111
N_CORES = 8
N_SHARD = N // N_CORES  # 4 batches per core
ROWS = N_SHARD * C  # 2048 rows per core
FILL = float(M)

_NC_CACHE = {}
LAST_RESULTS = None  # test harness introspection: last BassKernelResults
LAST_SYNCED = False  # True if the synced fallback program produced the output


def _build_nc(synced: bool = False):
    """Emit the per-core Bass program: fill 2048*4111 f32 elements with 16.0.

    The shard is declared as one [128, 65776] DRAM tensor (the linear
    buffer reshapes to (4, 512, 4111) on the host; every element is the
    same constant so element order is irrelevant). A [128, 176] SBUF tile
    is memset (Pool and DVE split the [0:128] region that gates DMA A;
    Pool then fills [128:176]), and two SP dma_starts with stride-0
    (broadcast) source APs replicate it across the free dim: DMA A covers
    the first 1536 columns with 512 B descriptors, DMA B the remaining
    64240 with 704 B 64B-aligned descriptors -- ~33.7 MB of pure HBM
    writes, zero HBM reads, a single HWDGE queue.

    synced=False (default): both DMAs increment dma_sem (codegen-mandated)
    but no instruction waits on it; the engines halt while the ~93.5 us of
    queued writes drain. Host readback (PJRT round-trip, ms-scale) trails
    the drain by >1000x and all output is verified host-side. Preamble IR
    surgery applies (see _strip_preamble). synced=True is the fully
    vanilla program with a classical completion wait, kept as fallback.
    """
    nc = bass.Bass(monotonic_sem_count=0)
    P = 128
    cols = (ROWS // P) * L  # 65776 f32 per partition row
    W0 = 176  # tile width; DMA B replicates [0:176] via a stride-0 AP

    if synced:
        # Vanilla fallback: full preamble, one memset, bulk + remainder
        # DMAs, classical completion wait. No IR surgery.
        reps = cols // W0  # 373
        rem = cols - reps * W0  # 128 columns
        out = nc.dram_tensor(
            "out", [P, cols], mybir.dt.float32, kind="ExternalOutput"
        )
        vsem = nc.semaphore("vsem").__enter__()
        dma_sem = nc.semaphore("dma_sem").__enter__()
        src_t = nc.sbuf_tensor("src", [P, W0], mybir.dt.float32).__enter__()
        src = src_t[:].rearrange("p (a w) -> p a w", a=1).broadcast_to(
            [P, reps, W0]
        )
        dst = out[:, : reps * W0].rearrange("p (r w) -> p r w", r=reps)

        with nc.Block() as block:

            @block.gpsimd
            def _(g):
                g.memset(src_t[:], FILL).then_inc(vsem, 1)

            @block.sync
            def _(s):
                bulk = s.dma_start(out=dst, in_=src)
                bulk._wait_ge(vsem, 1)
                last = s.dma_start(out=out[:, reps * W0 :], in_=src_t[:, :rem])
                bulk.then_inc(dma_sem, 16)
                last.then_inc(dma_sem, 16)
                s.wait_ge(dma_sem, 32)
        return nc

    # Aggressive program, emitted block-less straight into main.
    # DMA A (512 B descriptors over the first CA columns) sources the
    # tile's [0:128] region, which is memset by TWO engines in parallel
    # (Pool [0:90], DVE [90:128] -- the split balances Pool's Q7 launch
    # against DVE's access latency), releasing A's HWDGE generation
    # ~100 ns earlier than a single full-tile memset would. DMA B (704 B
    # descriptors over the rest) generates during A's transfer, and CA
    # is sized so A's transfer (~2.2 us) covers B's DGE->DMA delay: B
    # starts with zero gap. Column split solves 128a + 176b = 65776
    # (a=12, b=365).
    CA = 1536
    CB = cols - CA  # 64240 = 176 * 365
    S1 = 90  # Pool/DVE split point inside the [0:128] region
    a_reps, b_reps = CA // 128, CB // W0
    out = nc.dram_tensor("out", [P, cols], mybir.dt.float32, kind="ExternalOutput")
    vsem = nc.semaphore("vsem").__enter__()
    dma_sem = nc.semaphore("dma_sem").__enter__()
    src_t = nc.sbuf_tensor("src", [P, W0], mybir.dt.float32).__enter__()

    nc.gpsimd.memset(src_t[:, :S1], FILL).then_inc(vsem, 1)
    nc.vector.memset(src_t[:, S1:128], FILL).then_inc(vsem, 1)
    nc.gpsimd.memset(src_t[:, 128:], FILL).then_inc(vsem, 1)

    srcA = src_t[:, :128].rearrange("p (a w) -> p a w", a=1).broadcast_to(
        [P, a_reps, 128]
    )
    dstA = out[:, :CA].rearrange("p (r w) -> p r w", r=a_reps)
    srcB = src_t[:].rearrange("p (a w) -> p a w", a=1).broadcast_to(
        [P, b_reps, W0]
    )
    dstB = out[:, CA:].rearrange("p (r w) -> p r w", r=b_reps)

    dma_a = nc.sync.dma_start(out=dstA, in_=srcA)
    dma_a._wait_ge(vsem, 2)  # fused: SEQ decode overlaps the memsets
    dma_b = nc.sync.dma_start(out=dstB, in_=srcB)
    dma_b._wait_ge(vsem, 3)  # satisfied long before B's SEQ slot opens
    # walrus codegen requires >=1 sem update per HWDGE DMA
    # (CoreV2GenImpl reads updates.front()), so both carry one.
    dma_a.then_inc(dma_sem, 16)
    dma_b.then_inc(dma_sem, 16)

    _strip_preamble(nc)
    return nc


def _strip_preamble(nc):
    """IR surgery on the emitted program (unsynced variant only; the synced
    fallback stays fully vanilla). Three cuts, each verified on hardware
    (repeated full-output checks) and in TimelineSim:

    1. The four const-AP preamble memsets ([128, 1] tiles of 0.0/1.0/
       bf16-1.0/u8-127) -- never read by this program. Our own [128, 176]
       fill memset also lives in main and is kept (last-dim count > 1).
    2. The ENTIRE preamble barrier, including Pool's dma_reset/sem_clear
       drain and every engine's gather/release EventSemaphore. The
       barrier's only job for this program was guaranteeing vsem's
       initial value, and that guard is redundant: (a) on the first
       execution all semaphores must already be zero -- vanilla bass's
       own barrier correctness depends on that runtime guarantee (barrier
       sems are never cleared in-program either); (b) on re-execution a
       stale vsem>=1 lets SP's DMA skip the wait, but SBUF still holds
       16.0 from the previous run, so the output is correct either way
       (and nothing ever waits on dma_sem, so its accumulation across
       runs is harmless). The Pool->SP data ordering itself is carried by
       vsem, not the barrier.
    3. All RegisterMove preamble init (zero/bcreg scratch registers):
       no instruction in this program accesses any register
       (regs_accessed is empty on every DMACopy/Memset/EventSemaphore),
       and bcregs only matter for bounds-checked dynamic DMAs.

    The program is emitted block-less, so there is no end-of-block
    barrier either: engines halt right after their last instruction and
    the queued DMA writes drain (the verified-safe pattern). Net: the
    whole program is [call, memset x3, dmacopy x2] and everything before
    DMA A's HWDGE generation is the ~260 ns dual-engine memset chain.
    Verified on hardware with repeated full-output runs, both
    fresh-process and repeated-execution (stale-sem) paths.
    """
    DROP = ("EngineType.Activation", "EngineType.PE")  # DVE runs a memset
    for bb in nc.m.functions[0].blocks:
        kept = []
        for i in bb.instructions:
            tn = type(i).__name__
            if str(i.engine) in DROP:
                continue
            if tn == "InstRegisterMove":
                continue
            if tn in ("InstDrain", "InstEventSemaphore"):
                continue  # both barriers + dma_reset (see docstring)
            if tn == "InstMemset" and i.outs[0].ap[-1][1] == 1:
                continue  # const-AP tile; ours are [128, 90/38/48]
            kept.append(i)
        bb.instructions = kept


def _get_nc(synced: bool):
    if synced not in _NC_CACHE:
        _NC_CACHE[synced] = _build_nc(synced)
    return _NC_CACHE[synced]


def _run(nc):
    core_ids = list(range(N_CORES))
    in_maps = [{} for _ in core_ids]
    try:
        return run_bass_kernel_spmd(nc, in_maps, core_ids)
    except ModuleNotFoundError:
        # BASS_TRACE set but the axon NTFF profile hook isn't installed
        # in this container; retry with tracing hard-disabled.
        os.environ["BASS_NEVER_TRACE"] = "1"
        return run_bass_kernel_spmd(nc, in_maps, core_ids)


def kernel(x: np.ndarray, complex_weight: np.ndarray) -> np.ndarray:
    global LAST_RESULTS, LAST_SYNCED

    core_ids = list(range(N_CORES))
    last_err = None
    res = None
    # One unsynced attempt; any failure (compile, tunnel, self-check) falls
    # back to the fully-synced vanilla program for the remaining attempts.
    for attempt, synced in enumerate([False, True, True, True]):
        if attempt:
            time.sleep(30)  # axon terminal outages observed to self-recover
        try:
            res = _run(_get_nc(synced))
        except Exception as e:  # transient tunnel/device failure
            last_err = e
            res = None
            continue
        sample = [res.results[c]["out"][::37, ::1013] for c in core_ids]
        if all((s == FILL).all() for s in sample):
            LAST_SYNCED = synced
            break
        last_err = RuntimeError("device output failed sampled self-check")
        res = None
    else:
        raise last_err
    LAST_RESULTS = res

    shards = [res.results[c]["out"].reshape(N_SHARD, C, L) for c in core_ids]
    out = np.concatenate(shards, axis=0)
    return np.ascontiguousarray(out, dtype=np.float32)
